# revision 1
# baseline (speedup 1.0000x reference)
"""Causal CoreAttention kernel for Trainium2 (Bass/Tile), 8-core SPMD.

Problem: B=2, H=16, S=2048, D=128 fp32 causal attention.
Sharding: B*H=32 heads -> 4 heads per core across 8 cores.

Per-head algorithm (S^T layout, no transposes of P):
  - Load Q,K,V natural [2048,128]; PE-transpose Q,K -> Q^T,K^T [128,2048]
    (fp32 transpose, DVE evacuates PSUM rounding to float32r).
  - For each k-tile kt: S^T[kt,q] = K^T[:,kt].T @ Q^T[:,q] (f32r matmul,
    causal-exact strips q >= 128*kt), ACT exp (scale=1/sqrt(D)) evacuates
    PSUM -> bf16 P^T tiles; diagonal tile masked with upper-tri 0/1 mask.
  - For each q-tile qt: O_aug[qt] = sum_kt P^T[kt,qt].T @ [V[kt] | 1]
    (bf16, N=129; col 128 accumulates the softmax denominator).
  - Normalize per-partition with DVE reciprocal + tensor_scalar, DMA out.
"""
import math

import numpy as np

import concourse.bass as bass
import concourse.mybir as mybir
import concourse.tile as tile
from concourse.bass_utils import run_bass_kernel_spmd
from concourse.masks import make_identity, make_upper_triangular

B, H, S, D = 2, 16, 2048, 128
NCORES = 8
HPC = (B * H) // NCORES          # heads per core
NT = S // 128                    # 16 q/k tiles per head
SCALE = 1.0 / math.sqrt(D)

MAX_WAITS = 1  # walrus TRN2 encodes at most 1 sync-wait per instruction


def _split_waits(nc):
    """Tile emits >1 sem-wait on some instructions; hoist extras onto NoOps
    inserted just before, on the same (in-order) engine."""
    for f in nc.m.functions:
        for bb in f.blocks:
            insts = bb.instructions
            out = []
            changed = False
            for inst in insts:
                si = inst.sync_info
                if si is not None and len(si.on_wait) > MAX_WAITS:
                    waits = list(si.on_wait)
                    extra, keep = waits[:-MAX_WAITS], waits[-MAX_WAITS:]
                    for j in range(0, len(extra), MAX_WAITS):
                        nop = mybir.InstNoOp(
                            name=f"{inst.name}-ws{j}", engine=inst.engine)
                        nop.sync_info = mybir.SyncInfo(
                            on_wait=extra[j:j + MAX_WAITS], on_update=[])
                        out.append(nop)
                    inst.sync_info = mybir.SyncInfo(
                        on_wait=keep, on_update=list(si.on_update))
                    changed = True
                out.append(inst)
            if changed:
                insts[:] = out


def build_nc():
    fp32 = mybir.dt.float32
    f32r = mybir.dt.float32r
    bf16 = mybir.dt.bfloat16

    nc = bass.Bass("TRN2", target_bir_lowering=False)
    q = nc.dram_tensor("q", [HPC, S, D], fp32, kind="ExternalInput").ap()
    k = nc.dram_tensor("k", [HPC, S, D], fp32, kind="ExternalInput").ap()
    v = nc.dram_tensor("v", [HPC, S, D], fp32, kind="ExternalInput").ap()
    o = nc.dram_tensor("o", [HPC, S, D], fp32, kind="ExternalOutput").ap()

    # P^T strip offsets: strip kt covers q in [128*kt, 2048), stored packed
    off = []
    t = 0
    for kt in range(NT):
        off.append(t)
        t += S - 128 * kt
    pt_len = t  # 17408

    with tile.TileContext(nc) as tc:
        with tc.tile_pool(name="const", bufs=1) as constp, \
             tc.tile_pool(name="nat", bufs=3) as natp, \
             tc.tile_pool(name="qkT", bufs=2) as qktp, \
             tc.tile_pool(name="vaug", bufs=2) as vaugp, \
             tc.tile_pool(name="pt", bufs=2) as ptp, \
             tc.tile_pool(name="osb", bufs=2) as osbp, \
             tc.tile_pool(name="rc", bufs=2) as rcp, \
             tc.tile_pool(name="tr_ps", bufs=2, space="PSUM") as trps, \
             tc.tile_pool(name="qk_ps", bufs=2, space="PSUM") as qkps, \
             tc.tile_pool(name="pv_ps", bufs=2, space="PSUM") as pvps:

            ident = constp.tile([128, 128], fp32, tag="ident")
            make_identity(nc, ident[:])
            ltri = constp.tile([128, 128], bf16, tag="ltri")
            # keep P^T[k,q] where k <= q (partition <= free)
            make_upper_triangular(nc, ltri[:], val=1.0, diag=True)

            for h in range(HPC):
                # ---- load naturals ----
                qn = natp.tile([128, NT, 128], fp32, tag="qn")
                kn = natp.tile([128, NT, 128], fp32, tag="kn")
                vn = natp.tile([128, NT, 128], fp32, tag="vn")
                nc.gpsimd.dma_start(
                    qn[:], q[h].rearrange("(t p) d -> p t d", p=128))
                nc.gpsimd.dma_start(
                    kn[:], k[h].rearrange("(t p) d -> p t d", p=128))
                nc.gpsimd.dma_start(
                    vn[:], v[h].rearrange("(t p) d -> p t d", p=128))

                # ---- transpose Q,K -> [d, s] (f32r rounded) ----
                qT = qktp.tile([128, S], f32r, tag="qT")
                kT = qktp.tile([128, S], f32r, tag="kT")
                for src, dst in ((qn, qT), (kn, kT)):
                    for g in range(NT // 4):      # groups of 4 tiles per bank
                        pst = trps.tile([128, 512], fp32, tag="tr")
                        for j in range(4):
                            nc.tensor.transpose(
                                pst[:, j * 128:(j + 1) * 128],
                                src[:, g * 4 + j, :], ident[:])
                        nc.vector.tensor_copy(
                            dst[:, g * 512:(g + 1) * 512], pst[:])

                # ---- V -> bf16 with ones column (129 wide, pad to 130) ----
                va = vaugp.tile([128, NT, 130], bf16, tag="va")
                nc.vector.memset(va[:], 1.0)
                for t_ in range(NT):
                    nc.vector.tensor_copy(va[:, t_, 0:128], vn[:, t_, :])

                # ---- QK^T strips + exp ----
                ptall = ptp.tile([128, pt_len], bf16, tag="pt")
                for kt in range(NT):
                    q0 = kt * 128
                    pieces = [(q0, 1024), (1024, 2048)] if q0 < 1024 \
                        else [(q0, 2048)]
                    for (a, b) in pieces:
                        ln = b - a
                        ps = qkps.tile([128, 1024], fp32, tag="qk")
                        for c0 in range(0, ln, 512):
                            c1 = min(c0 + 512, ln)
                            nc.tensor.matmul(
                                ps[:, c0:c1],
                                kT[:, q0:q0 + 128],
                                qT[:, a + c0:a + c1],
                                start=True, stop=True)
                        nc.scalar.activation(
                            ptall[:, off[kt] + (a - q0):off[kt] + (b - q0)],
                            ps[:, 0:ln],
                            mybir.ActivationFunctionType.Exp,
                            scale=SCALE)
                    # zero the below-diagonal triangle of the diagonal tile
                    nc.vector.tensor_mul(
                        ptall[:, off[kt]:off[kt] + 128],
                        ptall[:, off[kt]:off[kt] + 128],
                        ltri[:])

                # ---- PV with fused denominator ----
                osb = osbp.tile([128, NT, 128], fp32, tag="osb")
                rc = rcp.tile([128, NT], fp32, tag="rc")
                for qt in range(NT):
                    po = pvps.tile([128, 129], fp32, tag="pv")
                    for kt in range(qt + 1):
                        nc.tensor.matmul(
                            po[:],
                            ptall[:, off[kt] + (qt - kt) * 128:
                                  off[kt] + (qt - kt) * 128 + 128],
                            va[:, kt, 0:129],
                            start=(kt == 0), stop=(kt == qt))
                    nc.vector.reciprocal(rc[:, qt:qt + 1], po[:, 128:129])
                    nc.vector.tensor_scalar_mul(
                        osb[:, qt, :], po[:, 0:128], rc[:, qt:qt + 1])

                nc.gpsimd.dma_start(
                    o[h].rearrange("(t p) d -> p t d", p=128), osb[:])

    _split_waits(nc)
    return nc


_NC = None


def kernel(query_states, key_states, value_states):
    global _NC
    qf = np.ascontiguousarray(
        np.asarray(query_states, dtype=np.float32).reshape(B * H, S, D))
    kf = np.ascontiguousarray(
        np.asarray(key_states, dtype=np.float32).reshape(B * H, S, D))
    vf = np.ascontiguousarray(
        np.asarray(value_states, dtype=np.float32).reshape(B * H, S, D))

    if _NC is None:
        _NC = build_nc()

    in_maps = [
        {"q": qf[i * HPC:(i + 1) * HPC],
         "k": kf[i * HPC:(i + 1) * HPC],
         "v": vf[i * HPC:(i + 1) * HPC]}
        for i in range(NCORES)
    ]
    res = run_bass_kernel_spmd(_NC, in_maps, core_ids=list(range(NCORES)))
    out = np.concatenate([res.results[i]["o"] for i in range(NCORES)], axis=0)
    return out.reshape(B, H, S, D)


# revision 6
# speedup vs baseline: 764.4662x; 764.4662x over previous
"""Causal CoreAttention kernel for Trainium2 (Bass/Tile), 8-core SPMD.

Problem: B=2, H=16, S=2048, D=128 fp32 causal attention.
Sharding: B*H=32 heads -> 4 heads per core across 8 cores.

Per-head algorithm (S^T layout, no transposes of P):
  - Load Q,K,V natural [2048,128]; PE-transpose Q,K -> Q^T,K^T [128,2048]
    (fp32 transpose, DVE evacuates PSUM rounding to float32r).
  - For each k-tile kt: S^T[kt,q] = K^T[:,kt].T @ Q^T[:,q] (f32r matmul,
    causal-exact strips q >= 128*kt), ACT exp (scale=1/sqrt(D)) evacuates
    PSUM -> bf16 P^T tiles; diagonal tile masked with upper-tri 0/1 mask.
  - For each q-tile qt: O_aug[qt] = sum_kt P^T[kt,qt].T @ [V[kt] | 1]
    (bf16, N=129; col 128 accumulates the softmax denominator).
  - Normalize per-partition with DVE reciprocal + tensor_scalar, DMA out.
"""
import math

import numpy as np

import concourse.bass as bass
import concourse.mybir as mybir
import concourse.tile as tile
from concourse.bass_utils import run_bass_kernel_spmd
from concourse.masks import make_identity, make_upper_triangular

B, H, S, D = 2, 16, 2048, 128
NCORES = 8
HPC = (B * H) // NCORES          # heads per core
NT = S // 128                    # 16 q/k tiles per head
SCALE = 1.0 / math.sqrt(D)

MAX_WAITS = 1  # walrus TRN2 encodes at most 1 sync-wait per instruction


def _split_waits(nc):
    """Tile emits >1 sem-wait on some instructions; hoist extras onto NoOps
    inserted just before, on the same (in-order) engine."""
    for f in nc.m.functions:
        for bb in f.blocks:
            insts = bb.instructions
            out = []
            changed = False
            for inst in insts:
                si = inst.sync_info
                if si is not None and len(si.on_wait) > MAX_WAITS:
                    waits = list(si.on_wait)
                    extra, keep = waits[:-MAX_WAITS], waits[-MAX_WAITS:]
                    for j in range(0, len(extra), MAX_WAITS):
                        nop = mybir.InstNoOp(
                            name=f"{inst.name}-ws{j}", engine=inst.engine)
                        nop.sync_info = mybir.SyncInfo(
                            on_wait=extra[j:j + MAX_WAITS], on_update=[])
                        out.append(nop)
                    inst.sync_info = mybir.SyncInfo(
                        on_wait=keep, on_update=list(si.on_update))
                    changed = True
                out.append(inst)
            if changed:
                insts[:] = out


def build_nc(qk_piece=1024, mask_on_gpsimd=False, nat_bufs=2, interleave=False,
             tr_bufs=2, pv_bufs=2, pt_bufs=2):
    fp32 = mybir.dt.float32
    f32r = mybir.dt.float32r
    bf16 = mybir.dt.bfloat16

    nc = bass.Bass("TRN2", target_bir_lowering=False)
    q = nc.dram_tensor("q", [HPC, S, D], fp32, kind="ExternalInput").ap()
    k = nc.dram_tensor("k", [HPC, S, D], fp32, kind="ExternalInput").ap()
    v = nc.dram_tensor("v", [HPC, S, D], fp32, kind="ExternalInput").ap()
    o = nc.dram_tensor("o", [HPC, S, D], fp32, kind="ExternalOutput").ap()

    # P^T strip offsets: strip kt covers q in [128*kt, 2048), stored packed
    off = []
    t = 0
    for kt in range(NT):
        off.append(t)
        t += S - 128 * kt
    pt_len = t  # 17408

    with tile.TileContext(nc) as tc:
        with tc.tile_pool(name="const", bufs=1) as constp, \
             tc.tile_pool(name="nat", bufs=nat_bufs) as natp, \
             tc.tile_pool(name="qkT", bufs=2) as qktp, \
             tc.tile_pool(name="vaug", bufs=2) as vaugp, \
             tc.tile_pool(name="pt", bufs=pt_bufs) as ptp, \
             tc.tile_pool(name="osb", bufs=2) as osbp, \
             tc.tile_pool(name="rc", bufs=2) as rcp, \
             tc.tile_pool(name="tr_ps", bufs=tr_bufs, space="PSUM") as trps, \
             tc.tile_pool(name="qk_ps", bufs=2, space="PSUM") as qkps, \
             tc.tile_pool(name="pv_ps", bufs=pv_bufs, space="PSUM") as pvps:

            ident = constp.tile([128, 128], fp32, tag="ident")
            make_identity(nc, ident[:])
            ltri = constp.tile([128, 128], bf16, tag="ltri")
            # keep P^T[k,q] where k <= q (partition <= free)
            make_upper_triangular(nc, ltri[:], val=1.0, diag=True)

            for h in range(HPC):
                # ---- load naturals ----
                qn = natp.tile([128, NT, 128], fp32, tag="qn")
                kn = natp.tile([128, NT, 128], fp32, tag="kn")
                vn = natp.tile([128, NT, 128], fp32, tag="vn")
                nc.gpsimd.dma_start(
                    qn[:], q[h].rearrange("(t p) d -> p t d", p=128))
                nc.gpsimd.dma_start(
                    kn[:], k[h].rearrange("(t p) d -> p t d", p=128))
                nc.gpsimd.dma_start(
                    vn[:], v[h].rearrange("(t p) d -> p t d", p=128))

                # ---- transpose Q,K -> [d, s] (f32r rounded) ----
                qT = qktp.tile([128, S], f32r, tag="qT")
                kT = qktp.tile([128, S], f32r, tag="kT")
                for src, dst in ((qn, qT), (kn, kT)):
                    for g in range(NT // 4):      # groups of 4 tiles per bank
                        pst = trps.tile([128, 512], fp32, tag="tr")
                        for j in range(4):
                            nc.tensor.transpose(
                                pst[:, j * 128:(j + 1) * 128],
                                src[:, g * 4 + j, :], ident[:])
                        nc.vector.tensor_copy(
                            dst[:, g * 512:(g + 1) * 512], pst[:])

                # ---- V -> bf16 with ones column (129 wide, pad to 130) ----
                va = vaugp.tile([128, NT, 130], bf16, tag="va")
                nc.vector.memset(va[:], 1.0)
                for t_ in range(NT):
                    nc.vector.tensor_copy(va[:, t_, 0:128], vn[:, t_, :])

                # ---- QK^T strips + exp ----
                ptall = ptp.tile([128, pt_len], bf16, tag="pt")
                osb = osbp.tile([128, NT, 128], fp32, tag="osb")
                rc = rcp.tile([128, NT], fp32, tag="rc")

                def emit_pv(qt):
                    po = pvps.tile([128, 129], fp32, tag="pv")
                    for kt in range(qt + 1):
                        nc.tensor.matmul(
                            po[:],
                            ptall[:, off[kt] + (qt - kt) * 128:
                                  off[kt] + (qt - kt) * 128 + 128],
                            va[:, kt, 0:129],
                            start=(kt == 0), stop=(kt == qt))
                    nc.vector.reciprocal(rc[:, qt:qt + 1], po[:, 128:129])
                    nc.vector.tensor_scalar_mul(
                        osb[:, qt, :], po[:, 0:128], rc[:, qt:qt + 1])

                for kt in range(NT):
                    q0 = kt * 128
                    bnds = [q0] + [x for x in range(qk_piece, S, qk_piece)
                                   if x > q0] + [S]
                    pieces = list(zip(bnds[:-1], bnds[1:]))
                    for (a, b) in pieces:
                        ln = b - a
                        ps = qkps.tile([128, qk_piece], fp32, tag="qk")
                        for c0 in range(0, ln, 512):
                            c1 = min(c0 + 512, ln)
                            nc.tensor.matmul(
                                ps[:, c0:c1],
                                kT[:, q0:q0 + 128],
                                qT[:, a + c0:a + c1],
                                start=True, stop=True)
                        nc.scalar.activation(
                            ptall[:, off[kt] + (a - q0):off[kt] + (b - q0)],
                            ps[:, 0:ln],
                            mybir.ActivationFunctionType.Exp,
                            scale=SCALE)
                    # zero the below-diagonal triangle of the diagonal tile
                    if mask_on_gpsimd:
                        nc.gpsimd.affine_select(
                            out=ptall[:, off[kt]:off[kt] + 128],
                            in_=ptall[:, off[kt]:off[kt] + 128],
                            compare_op=mybir.AluOpType.is_ge,
                            fill=0.0, base=0,
                            pattern=[[1, 128]], channel_multiplier=-1)
                    else:
                        nc.vector.tensor_mul(
                            ptall[:, off[kt]:off[kt] + 128],
                            ptall[:, off[kt]:off[kt] + 128],
                            ltri[:])
                    if interleave:
                        emit_pv(kt)

                if not interleave:
                    for qt in range(NT):
                        emit_pv(qt)

                nc.gpsimd.dma_start(
                    o[h].rearrange("(t p) d -> p t d", p=128), osb[:])

    _split_waits(nc)
    return nc


_NC = None


def kernel(query_states, key_states, value_states):
    global _NC
    qf = np.ascontiguousarray(
        np.asarray(query_states, dtype=np.float32).reshape(B * H, S, D))
    kf = np.ascontiguousarray(
        np.asarray(key_states, dtype=np.float32).reshape(B * H, S, D))
    vf = np.ascontiguousarray(
        np.asarray(value_states, dtype=np.float32).reshape(B * H, S, D))

    if _NC is None:
        _NC = build_nc()

    in_maps = [
        {"q": qf[i * HPC:(i + 1) * HPC],
         "k": kf[i * HPC:(i + 1) * HPC],
         "v": vf[i * HPC:(i + 1) * HPC]}
        for i in range(NCORES)
    ]
    res = run_bass_kernel_spmd(_NC, in_maps, core_ids=list(range(NCORES)))
    out = np.concatenate([res.results[i]["o"] for i in range(NCORES)], axis=0)
    return out.reshape(B, H, S, D)
